# revision 1
# baseline (speedup 1.0000x reference)
"""RBF kernel feature map: out[b, r] = exp(-||x[b] - refs[r]||^2).

Computed via the GEMM expansion on 8 NeuronCores, data-parallel over the
batch dim of x (2048 rows per core), refs replicated.

Per-core device kernel, one K=68 matmul per [128, 512] output tile:
    psum[b, r] = sum_d x[b,d]*refs[r,d] - 0.5*r_sq[r] - 0.5*x_sq[b]
    out[b, r]  = exp(2 * psum[b, r])                     (ScalarE Exp)

The contraction packs all the norm terms into 4 extra K rows (vs ones-rows
on the opposite side). Each norm term is split hi (bf16-representable) +
lo (remainder) so the full-rate fp32r matmul path keeps the ~30..300
magnitude norm terms accurate even if fp32r quantizes inputs to ~tf32
internally.

Uses bacc.Bacc (not raw bass.Bass): TRN2 instructions carry at most one
semaphore wait, and Bacc.compile()'s generate_event_semaphores pass
legalizes the multi-wait instructions Tile emits.
"""

import numpy as np

N_CORES = 8
B, D, R = 16384, 64, 2048
B_SHARD = B // N_CORES  # 2048
K = D + 4  # 64 data rows + r_sq hi/lo + x_sq hi/lo rows
BT = 128  # batch rows per tile (PSUM partition dim)
RC = 512  # refs cols per matmul (max fp32 moving free dim)

MM_DT = "float32r"  # full-rate fp32 matmul mode; "float32" = 4x slower, exact


def _build_nc():
    from contextlib import ExitStack

    import concourse.tile as tile
    from concourse import bacc, mybir

    mm_dt = getattr(mybir.dt, MM_DT)
    f32 = mybir.dt.float32

    nc = bacc.Bacc(None)
    # x-shard transpose and refs transpose concatenated along the free dim so
    # ONE DMA loads both matmul operands.
    inT_aug = nc.declare_dram_parameter(
        "inT_aug", [K, B_SHARD + R], mm_dt, isOutput=False
    )
    out = nc.declare_dram_parameter("out", [B_SHARD, R], f32, isOutput=True)

    n_bt = B_SHARD // BT
    n_rc = R // RC

    with tile.TileContext(nc) as tc, ExitStack() as ctx:
        consts = ctx.enter_context(tc.tile_pool(name="consts", bufs=1))
        outs = ctx.enter_context(tc.tile_pool(name="outs", bufs=8))
        psums = ctx.enter_context(tc.tile_pool(name="psums", bufs=8, space="PSUM"))

        # SWDGE (gpsimd) for the input load: the HWDGE path moved this
        # 68-partition/16KB-line transfer on ~2 SDMA engines (~55 GB/s,
        # ~20us stall before the first matmul); SWDGE sprays descriptors
        # across all 16 engines.
        in_sb = consts.tile([K, B_SHARD + R], mm_dt)
        nc.gpsimd.dma_start(out=in_sb, in_=inT_aug[:, :])

        for bt in range(n_bt):
            out_sb = outs.tile([BT, R], f32)
            for rc in range(n_rc):
                ps = psums.tile([BT, RC], mybir.dt.float32)
                nc.tensor.matmul(
                    ps,
                    lhsT=in_sb[:, bt * BT : (bt + 1) * BT],
                    rhs=in_sb[:, B_SHARD + rc * RC : B_SHARD + (rc + 1) * RC],
                    start=True,
                    stop=True,
                )
                nc.scalar.activation(
                    out_sb[:, rc * RC : (rc + 1) * RC],
                    ps,
                    mybir.ActivationFunctionType.Exp,
                    bias=0.0,
                    scale=2.0,
                )
            nc.sync.dma_start(out=out[bt * BT : (bt + 1) * BT, :], in_=out_sb)

    nc.compile()
    return nc


def _hi_lo(v):
    """Split fp64 vector into bf16-representable hi + fp32 remainder lo."""
    import ml_dtypes

    hi = v.astype(np.float32).astype(ml_dtypes.bfloat16).astype(np.float32)
    lo = (v - hi).astype(np.float32)
    return hi, lo


def make_in_maps(x, refs):
    """Host-side prep: shard/transpose x, pack norm terms as extra K rows."""
    x = np.ascontiguousarray(x, dtype=np.float32)
    refs = np.ascontiguousarray(refs, dtype=np.float32)

    r_hi, r_lo = _hi_lo(0.5 * (refs.astype(np.float64) ** 2).sum(axis=1))
    x_sq = 0.5 * (x.astype(np.float64) ** 2).sum(axis=1)  # [B]

    in_maps = []
    for c in range(N_CORES):
        sl = slice(c * B_SHARD, (c + 1) * B_SHARD)
        x_hi, x_lo = _hi_lo(x_sq[sl])
        inT_aug = np.empty((K, B_SHARD + R), np.float32)
        inT_aug[:D, :B_SHARD] = x[sl].T
        inT_aug[D, :B_SHARD] = 1.0
        inT_aug[D + 1, :B_SHARD] = 1.0
        inT_aug[D + 2, :B_SHARD] = -x_hi
        inT_aug[D + 3, :B_SHARD] = -x_lo
        inT_aug[:D, B_SHARD:] = refs.T
        inT_aug[D, B_SHARD:] = -r_hi
        inT_aug[D + 1, B_SHARD:] = -r_lo
        inT_aug[D + 2, B_SHARD:] = 1.0
        inT_aug[D + 3, B_SHARD:] = 1.0
        in_maps.append({"inT_aug": inT_aug})
    return in_maps


_NC_CACHE = None


def get_nc():
    global _NC_CACHE
    if _NC_CACHE is None:
        _NC_CACHE = _build_nc()
    return _NC_CACHE


def kernel(x, refs):
    from concourse.bass_utils import run_bass_kernel_spmd

    in_maps = make_in_maps(x, refs)
    res = run_bass_kernel_spmd(
        get_nc(), in_maps, core_ids=list(range(N_CORES))
    ).results
    return np.concatenate([res[c]["out"] for c in range(N_CORES)], axis=0)



# revision 2
# speedup vs baseline: 1.4376x; 1.4376x over previous
"""RBF kernel feature map: out[b, r] = exp(-||x[b] - refs[r]||^2).

Computed via the GEMM expansion on 8 NeuronCores, data-parallel over the
batch dim of x (2048 rows per core), refs replicated.

Per-core device kernel, per [128, 2048] output tile:
    psum[b, r] = sum_d x[b,d]*refs[r,d] - 0.5*r_sq[r] - 0.5*x_sq[b]
      (4 matmuls of N=512 side by side into one 4-bank PSUM tile)
    out[b, r]  = exp(2 * psum[b, r])   (ONE ScalarE Exp over N=2048)

The contraction packs all the norm terms into 4 extra K rows (vs ones-rows
on the opposite side). Each norm term is split hi (bf16-representable) +
lo (remainder) so the full-rate fp32r matmul path keeps the ~30..300
magnitude norm terms accurate even if fp32r quantizes inputs to ~tf32
internally.

Perf notes vs the v1 kernel (79.6us):
  - input DRAM param padded from [68, B+R] to [128, B+R]: a 68-partition
    DMA was served by only 4 SDMA engines (77 GB/s, 14.4us startup
    stall); 128 partitions spread over all 16 engines.
  - output stored as bf16 and upcast on the host: halves the 16 MiB/core
    store traffic that ran at the ~358 GB/s per-core HBM ceiling.
    exp() outputs are in [0, ~1e-13]; bf16 adds <=2e-3 relative error
    against a 2e-2 budget (measured baseline error 2.8e-3).
  - one Exp ACTIVATE per [128, 2048] PSUM region instead of four per
    [128, 512]: the ~170-350 cycle per-instruction bubble made 64 small
    ACTs cost 59.7us of ScalarE time; 16 big ones cost ~32us.

Uses bacc.Bacc (not raw bass.Bass): TRN2 instructions carry at most one
semaphore wait, and Bacc.compile()'s generate_event_semaphores pass
legalizes the multi-wait instructions Tile emits.
"""

import numpy as np

N_CORES = 8
B, D, R = 16384, 64, 2048
B_SHARD = B // N_CORES  # 2048
K = D + 4  # 64 data rows + r_sq hi/lo + x_sq hi/lo rows
KP = 128  # padded partition count so the input DMA uses all 16 SDMA engines
BT = 128  # batch rows per tile (PSUM partition dim)
RC = 512  # refs cols per matmul (max fp32 moving free dim)

MM_DT = "float32r"  # full-rate fp32 matmul mode; "float32" = 4x slower, exact


def _build_nc():
    from contextlib import ExitStack

    import concourse.tile as tile
    from concourse import bacc, mybir

    mm_dt = getattr(mybir.dt, MM_DT)
    f32 = mybir.dt.float32
    bf16 = mybir.dt.bfloat16

    nc = bacc.Bacc(None)
    # x-shard transpose and refs transpose concatenated along the free dim so
    # ONE DMA loads both matmul operands.
    inT_aug = nc.declare_dram_parameter(
        "inT_aug", [KP, B_SHARD + R], mm_dt, isOutput=False
    )
    out = nc.declare_dram_parameter("out", [B_SHARD, R], bf16, isOutput=True)

    n_bt = B_SHARD // BT
    n_rc = R // RC

    with tile.TileContext(nc) as tc, ExitStack() as ctx:
        consts = ctx.enter_context(tc.tile_pool(name="consts", bufs=1))
        outs = ctx.enter_context(tc.tile_pool(name="outs", bufs=4))
        psums = ctx.enter_context(tc.tile_pool(name="psums", bufs=2, space="PSUM"))

        in_sb = consts.tile([KP, B_SHARD + R], mm_dt)
        nc.sync.dma_start(out=in_sb, in_=inT_aug[:, :])

        for bt in range(n_bt):
            ps = psums.tile([BT, R], mybir.dt.float32)
            for rc in range(n_rc):
                nc.tensor.matmul(
                    ps[:, rc * RC : (rc + 1) * RC],
                    lhsT=in_sb[0:K, bt * BT : (bt + 1) * BT],
                    rhs=in_sb[0:K, B_SHARD + rc * RC : B_SHARD + (rc + 1) * RC],
                    start=True,
                    stop=True,
                )
            out_sb = outs.tile([BT, R], bf16)
            nc.scalar.activation(
                out_sb,
                ps,
                mybir.ActivationFunctionType.Exp,
                bias=0.0,
                scale=2.0,
            )
            nc.sync.dma_start(out=out[bt * BT : (bt + 1) * BT, :], in_=out_sb)

    nc.compile()
    return nc


def _hi_lo(v):
    """Split fp64 vector into bf16-representable hi + fp32 remainder lo."""
    import ml_dtypes

    hi = v.astype(np.float32).astype(ml_dtypes.bfloat16).astype(np.float32)
    lo = (v - hi).astype(np.float32)
    return hi, lo


def make_in_maps(x, refs):
    """Host-side prep: shard/transpose x, pack norm terms as extra K rows."""
    x = np.ascontiguousarray(x, dtype=np.float32)
    refs = np.ascontiguousarray(refs, dtype=np.float32)

    r_hi, r_lo = _hi_lo(0.5 * (refs.astype(np.float64) ** 2).sum(axis=1))
    x_sq = 0.5 * (x.astype(np.float64) ** 2).sum(axis=1)  # [B]

    in_maps = []
    for c in range(N_CORES):
        sl = slice(c * B_SHARD, (c + 1) * B_SHARD)
        x_hi, x_lo = _hi_lo(x_sq[sl])
        inT_aug = np.zeros((KP, B_SHARD + R), np.float32)
        inT_aug[:D, :B_SHARD] = x[sl].T
        inT_aug[D, :B_SHARD] = 1.0
        inT_aug[D + 1, :B_SHARD] = 1.0
        inT_aug[D + 2, :B_SHARD] = -x_hi
        inT_aug[D + 3, :B_SHARD] = -x_lo
        inT_aug[:D, B_SHARD:] = refs.T
        inT_aug[D, B_SHARD:] = -r_hi
        inT_aug[D + 1, B_SHARD:] = -r_lo
        inT_aug[D + 2, B_SHARD:] = 1.0
        inT_aug[D + 3, B_SHARD:] = 1.0
        in_maps.append({"inT_aug": inT_aug})
    return in_maps


_NC_CACHE = None


def get_nc():
    global _NC_CACHE
    if _NC_CACHE is None:
        _NC_CACHE = _build_nc()
    return _NC_CACHE


def kernel(x, refs):
    from concourse.bass_utils import run_bass_kernel_spmd

    in_maps = make_in_maps(x, refs)
    res = run_bass_kernel_spmd(
        get_nc(), in_maps, core_ids=list(range(N_CORES))
    ).results
    return np.concatenate(
        [res[c]["out"].astype(np.float32) for c in range(N_CORES)], axis=0
    )
